# revision 4
# baseline (speedup 1.0000x reference)
"""AgentGNN (2x CGConv + BN + residual + ReLU) on 8 TRN2 NeuronCores.

Self-contained: takes FULL inputs, shards 8 samples/core (data parallel),
runs a Bass/Tile kernel via run_bass_kernel_spmd, gathers FULL output.

Math: edges are fully-connected per 64-node sample and e_ij = c_i - c_j,
so  z_ij @ W.T + b  separates into per-node terms:
    alpha_i = x_i@Wa.T + c_i@Wc.T + b     (target part,  Wa = W[:, :F])
    beta_j  = x_j@Wb.T - c_j@Wc.T         (source part,  Wb = W[:, F:2F])
    msg_ij  = sigmoid(alpha_i + beta_j) * softplus(gamma_i + delta_j)
    agg_i   = sum_j msg_ij - msg_ii       (dense 64x64 incl. diag, minus diag)
BN (over all 4096 nodes, per feature) needs one cross-core AllReduce of
[sum, sumsq] per layer.
"""

import numpy as np

N_SAMPLES = 64
N_AGENTS = 64
N = N_SAMPLES * N_AGENTS          # 4096
F = 128
EDIM = 2
BN_EPS = 1e-5
N_CORES = 8
S_PC = N_SAMPLES // N_CORES       # 8 samples per core
NODES_PC = S_PC * N_AGENTS        # 512 nodes per core
CHUNK_S = 2                       # samples per pairwise chunk
N_CHUNK = S_PC // CHUNK_S         # 4 chunks
GROUP = 2                         # chunks per ACT table-set group

_CACHE = {}


def _build_nc():
    from concourse import bacc, mybir
    from concourse.tile import TileContext
    from concourse.tile_rust import add_dep_helper

    f32 = mybir.dt.float32
    AF = mybir.ActivationFunctionType
    OP = mybir.AluOpType
    AX = mybir.AxisListType

    nc = bacc.Bacc(trn_type="TRN2", target_bir_lowering=False, debug=False,
                   num_devices=N_CORES)

    xT = nc.declare_dram_parameter("xT", [F, NODES_PC], f32, isOutput=False)
    c3 = nc.declare_dram_parameter("c3", [EDIM + 1, NODES_PC], f32, isOutput=False)
    wnames = ["WaT", "WbT", "VaT", "VbT"]            # [128,128] x-side lhsT
    cnames = ["Wc3a", "Wc3b", "Vc3g", "Vc3d"]        # [3,128] centers+bias lhsT
    params = {}
    for l in (1, 2):
        for n in wnames:
            params[f"{n}{l}"] = nc.declare_dram_parameter(f"{n}{l}", [F, F], f32, isOutput=False)
        for n in cnames:
            params[f"{n}{l}"] = nc.declare_dram_parameter(f"{n}{l}", [EDIM + 1, F], f32, isOutput=False)
        params[f"g{l}"] = nc.declare_dram_parameter(f"g{l}", [F, 1], f32, isOutput=False)
        params[f"be{l}"] = nc.declare_dram_parameter(f"be{l}", [F, 1], f32, isOutput=False)
    yT = nc.declare_dram_parameter("yT", [F, NODES_PC], f32, isOutput=True)

    # DRAM bounce buffers for the BN-stats AllReduce (one pair per layer)
    cc_in = {l: nc.dram_tensor(f"cc_in{l}", [F, 2], f32) for l in (1, 2)}
    cc_out = {l: nc.dram_tensor(f"cc_out{l}", [F, 2], f32, addr_space="Shared")
              for l in (1, 2)}

    with TileContext(nc) as tc:
        from contextlib import ExitStack
        with ExitStack() as ctx:
            io = ctx.enter_context(tc.tile_pool(name="io", bufs=1))
            wp = ctx.enter_context(tc.tile_pool(name="wp", bufs=1))
            node = ctx.enter_context(tc.tile_pool(name="node", bufs=1))
            pair = ctx.enter_context(tc.tile_pool(name="pair", bufs=2))
            psum = ctx.enter_context(tc.tile_pool(name="psum", bufs=1, space="PSUM"))
            small = ctx.enter_context(tc.tile_pool(name="small", bufs=1))

            # ---- load inputs & weights ----
            xt = io.tile([F, NODES_PC], f32, tag="xt")
            nc.sync.dma_start(xt[:], xT.ap()[:, :])
            c3t = io.tile([EDIM + 1, NODES_PC], f32, tag="c3t")
            nc.sync.dma_start(c3t[:], c3.ap()[:, :])
            wt = {}
            for l in (1, 2):
                for n in wnames:
                    t = wp.tile([F, F], f32, tag=f"{n}{l}")
                    nc.sync.dma_start(t[:], params[f"{n}{l}"].ap()[:, :])
                    wt[f"{n}{l}"] = t
                for n in cnames:
                    t = wp.tile([EDIM + 1, F], f32, tag=f"{n}{l}")
                    nc.sync.dma_start(t[:], params[f"{n}{l}"].ap()[:, :])
                    wt[f"{n}{l}"] = t
                for n in ("g", "be"):
                    t = wp.tile([F, 1], f32, tag=f"{n}{l}")
                    nc.sync.dma_start(t[:], params[f"{n}{l}"].ap()[:, :])
                    wt[f"{n}{l}"] = t

            def layer(l, x_in, x_out):
                # ---- node projections: alpha/beta/gamma/delta [128, 512] ----
                projs = {}
                for pi, (wx, wc) in enumerate(zip(wnames, cnames)):
                    ps = psum.tile([F, NODES_PC], f32, tag=f"ps{pi}")
                    nc.tensor.matmul(ps[:], wt[f"{wx}{l}"][:], x_in[:],
                                     start=True, stop=False)
                    nc.tensor.matmul(ps[:], wt[f"{wc}{l}"][:], c3t[:],
                                     start=False, stop=True)
                    sb = node.tile([F, NODES_PC], f32, tag=f"proj{pi}")
                    nc.scalar.copy(sb[:], ps[:])
                    projs[wx] = sb
                al, be_, ga, de = (projs[n] for n in wnames)

                # ---- diagonal (self-edge) messages, per node ----
                d1 = node.tile([F, NODES_PC], f32, tag="d1")
                nc.vector.tensor_tensor(d1[:], al[:], be_[:], op=OP.add)
                d2 = node.tile([F, NODES_PC], f32, tag="d2")
                nc.vector.tensor_tensor(d2[:], ga[:], de[:], op=OP.add)

                agg = node.tile([F, NODES_PC], f32, tag="agg")

                # ---- pairwise chunks, grouped for ACT table-set batching ----
                for g in range(N_CHUNK // GROUP):
                    cs = range(g * GROUP, (g + 1) * GROUP)
                    p1s, p2s = {}, {}
                    for c in cs:
                        n0 = c * CHUNK_S * N_AGENTS
                        sl = slice(n0, n0 + CHUNK_S * N_AGENTS)
                        a_bc = al[:, sl].rearrange("p (s i) -> p s i", s=CHUNK_S) \
                            .broadcast_to([F, CHUNK_S, N_AGENTS, N_AGENTS])
                        b_bc = be_[:, sl].rearrange("p (s o j) -> p s o j", s=CHUNK_S, o=1) \
                            .broadcast_to([F, CHUNK_S, N_AGENTS, N_AGENTS])
                        g_bc = ga[:, sl].rearrange("p (s i) -> p s i", s=CHUNK_S) \
                            .broadcast_to([F, CHUNK_S, N_AGENTS, N_AGENTS])
                        dl_bc = de[:, sl].rearrange("p (s o j) -> p s o j", s=CHUNK_S, o=1) \
                            .broadcast_to([F, CHUNK_S, N_AGENTS, N_AGENTS])
                        p1 = pair.tile([F, CHUNK_S, N_AGENTS, N_AGENTS], f32, tag="p1")
                        nc.vector.tensor_tensor(p1[:], a_bc, b_bc, op=OP.add)
                        p2 = pair.tile([F, CHUNK_S, N_AGENTS, N_AGENTS], f32, tag="p2")
                        nc.vector.tensor_tensor(p2[:], g_bc, dl_bc, op=OP.add)
                        p1s[c], p2s[c] = p1, p2
                    for c in cs:
                        nc.scalar.activation(p1s[c][:], p1s[c][:], AF.Sigmoid)
                    # softplus = ln(1 + exp(x)); this runtime's ACT tables
                    # have no softplus entry (b16 overlay replaced it)
                    for c in cs:
                        nc.scalar.activation(p2s[c][:], p2s[c][:], AF.Exp)
                    for c in cs:
                        nc.scalar.activation(p2s[c][:], p2s[c][:], AF.Ln, bias=1.0)
                    for c in cs:
                        nc.gpsimd.tensor_tensor(p1s[c][:], p1s[c][:], p2s[c][:],
                                                op=OP.mult)
                    for c in cs:
                        n0 = c * CHUNK_S * N_AGENTS
                        nc.vector.tensor_reduce(
                            agg[:, n0:n0 + CHUNK_S * N_AGENTS], p1s[c][:],
                            axis=AX.X, op=OP.add)

                # diagonal activations ride the last group's table sets
                nc.scalar.activation(d1[:], d1[:], AF.Sigmoid)
                nc.scalar.activation(d2[:], d2[:], AF.Exp)
                nc.scalar.activation(d2[:], d2[:], AF.Ln, bias=1.0)
                nc.gpsimd.tensor_tensor(d1[:], d1[:], d2[:], op=OP.mult)
                nc.vector.tensor_tensor(agg[:], agg[:], d1[:], op=OP.subtract)

                # ---- BN stats: per-feature sum & sumsq over this core ----
                ssum = small.tile([F, 1], f32, tag="ssum")
                nc.vector.tensor_reduce(ssum[:], agg[:], axis=AX.X, op=OP.add)
                trash = node.tile([F, NODES_PC], f32, tag="trash")
                ssq = small.tile([F, 1], f32, tag="ssq")
                nc.scalar.activation(trash[:], agg[:], AF.Square, accum_out=ssq[:])

                dsum = nc.sync.dma_start(cc_in[l].ap()[:, 0:1], ssum[:])
                dsq = nc.sync.dma_start(cc_in[l].ap()[:, 1:2], ssq[:])
                ar = nc.gpsimd.collective_compute(
                    "AllReduce", OP.add,
                    replica_groups=[list(range(N_CORES))],
                    ins=[cc_in[l].ap().opt()], outs=[cc_out[l].ap().opt()])
                add_dep_helper(ar.ins, dsum.ins, reason="cc reads cc_in")
                add_dep_helper(ar.ins, dsq.ins, reason="cc reads cc_in")
                gst = small.tile([F, 2], f32, tag="gst")
                din = nc.sync.dma_start(gst[:], cc_out[l].ap()[:, :])
                add_dep_helper(din.ins, ar.ins, reason="dma reads cc_out")

                # ---- BN apply + residual + relu ----
                mean = small.tile([F, 1], f32, tag="mean")
                nc.vector.tensor_scalar(mean[:], gst[:, 0:1], 1.0 / N, None, op0=OP.mult)
                ex2 = small.tile([F, 1], f32, tag="ex2")
                nc.vector.tensor_scalar(ex2[:], gst[:, 1:2], 1.0 / N, None, op0=OP.mult)
                var = small.tile([F, 1], f32, tag="var")
                nc.vector.tensor_tensor(var[:], mean[:], mean[:], op=OP.mult)
                nc.vector.tensor_tensor(var[:], ex2[:], var[:], op=OP.subtract)
                nc.vector.tensor_scalar(var[:], var[:], BN_EPS, None, op0=OP.add)
                lnv = small.tile([F, 1], f32, tag="lnv")
                nc.scalar.activation(lnv[:], var[:], AF.Ln, bias=0.0)
                rstd = small.tile([F, 1], f32, tag="rstd")
                nc.scalar.activation(rstd[:], lnv[:], AF.Exp, bias=0.0, scale=-0.5)
                scal = small.tile([F, 1], f32, tag="scal")
                nc.vector.tensor_tensor(scal[:], rstd[:], wt[f"g{l}"][:], op=OP.mult)
                shift = small.tile([F, 1], f32, tag="shift")
                nc.vector.tensor_tensor(shift[:], mean[:], scal[:], op=OP.mult)
                nc.vector.tensor_tensor(shift[:], wt[f"be{l}"][:], shift[:], op=OP.subtract)
                nc.vector.tensor_scalar(x_out[:], agg[:], scal[:, 0:1], shift[:, 0:1],
                                        op0=OP.mult, op1=OP.add)
                nc.vector.tensor_tensor(x_out[:], x_out[:], x_in[:], op=OP.add)
                nc.scalar.activation(x_out[:], x_out[:], AF.Relu)

            x1 = io.tile([F, NODES_PC], f32, tag="x1")
            layer(1, xt, x1)
            x2 = io.tile([F, NODES_PC], f32, tag="x2")
            layer(2, x1, x2)
            nc.sync.dma_start(yT.ap()[:, :], x2[:])

    nc.compile()
    return nc


def _get_nc():
    if "nc" not in _CACHE:
        _CACHE["nc"] = _build_nc()
    return _CACHE["nc"]


def _canonical_edge_ok(src, dst):
    idx = np.arange(N_AGENTS)
    rows = np.repeat(idx, N_AGENTS)
    cols = np.tile(idx, N_AGENTS)
    m = rows != cols
    rows, cols = rows[m], cols[m]
    offs = (np.arange(N_SAMPLES) * N_AGENTS)[:, None]
    csrc = (rows[None, :] + offs).ravel().astype(np.int64)
    cdst = (cols[None, :] + offs).ravel().astype(np.int64)
    if src.shape != csrc.shape:
        return False
    key = np.sort(src.astype(np.int64) * N + dst.astype(np.int64))
    ckey = np.sort(csrc * N + cdst)
    return bool(np.array_equal(key, ckey))


def _numpy_fallback(gnn_in, centers, src, dst, Ws_all):
    def sig(x):
        return 1.0 / (1.0 + np.exp(-x))

    def sp(x):
        return np.log1p(np.exp(-np.abs(x))) + np.maximum(x, 0.0)

    x = gnn_in.astype(np.float64)
    e = (centers[dst] - centers[src]).astype(np.float64)
    for (Wf, bf, Wsm, bs, g, be) in Ws_all:
        z = np.concatenate([x[dst], x[src], e], axis=-1)
        msg = sig(z @ Wf.T + bf) * sp(z @ Wsm.T + bs)
        agg = np.zeros_like(x)
        np.add.at(agg, dst, msg)
        mean = agg.mean(0)
        var = agg.var(0)
        agg = (agg - mean) / np.sqrt(var + BN_EPS) * g + be
        x = np.maximum(agg + x, 0.0)
    return x.astype(np.float32)


def _host_weights(Wf, bf, Ws, bs):
    """lhsT forms for the projection matmuls."""
    WaT = np.ascontiguousarray(Wf[:, :F].T)
    WbT = np.ascontiguousarray(Wf[:, F:2 * F].T)
    Wc = Wf[:, 2 * F:2 * F + EDIM].T           # [2, 128]
    Wc3a = np.ascontiguousarray(np.concatenate([Wc, bf[None, :]], 0))
    Wc3b = np.ascontiguousarray(np.concatenate([-Wc, np.zeros((1, F), np.float32)], 0))
    VaT = np.ascontiguousarray(Ws[:, :F].T)
    VbT = np.ascontiguousarray(Ws[:, F:2 * F].T)
    Vc = Ws[:, 2 * F:2 * F + EDIM].T
    Vc3g = np.ascontiguousarray(np.concatenate([Vc, bs[None, :]], 0))
    Vc3d = np.ascontiguousarray(np.concatenate([-Vc, np.zeros((1, F), np.float32)], 0))
    return WaT, WbT, Wc3a, Wc3b, VaT, VbT, Vc3g, Vc3d


def kernel(gnn_in, centers, src, dst,
           Wf1, bf1, Ws1, bs1, g1, be1,
           Wf2, bf2, Ws2, bs2, g2, be2,
           _trace=False, _tmpdir=None):
    gnn_in = np.ascontiguousarray(np.asarray(gnn_in, np.float32))
    centers = np.ascontiguousarray(np.asarray(centers, np.float32))
    src = np.asarray(src, np.int32)
    dst = np.asarray(dst, np.int32)
    args = [np.asarray(a, np.float32) for a in
            (Wf1, bf1, Ws1, bs1, g1, be1, Wf2, bf2, Ws2, bs2, g2, be2)]
    (Wf1, bf1, Ws1, bs1, g1, be1, Wf2, bf2, Ws2, bs2, g2, be2) = args

    if not _canonical_edge_ok(src, dst):
        import sys
        print("kernel.py: edge index is not block-fully-connected; numpy fallback",
              file=sys.stderr)
        return _numpy_fallback(gnn_in, centers, src, dst,
                               [(Wf1, bf1, Ws1, bs1, g1, be1),
                                (Wf2, bf2, Ws2, bs2, g2, be2)])

    from concourse import bass_utils

    nc = _get_nc()

    w1 = _host_weights(Wf1, bf1, Ws1, bs1)
    w2 = _host_weights(Wf2, bf2, Ws2, bs2)
    wmap = {}
    for l, w in ((1, w1), (2, w2)):
        for n, a in zip(("WaT", "WbT", "Wc3a", "Wc3b", "VaT", "VbT", "Vc3g", "Vc3d"), w):
            wmap[f"{n}{l}"] = a
    wmap["g1"] = np.ascontiguousarray(g1[:, None])
    wmap["be1"] = np.ascontiguousarray(be1[:, None])
    wmap["g2"] = np.ascontiguousarray(g2[:, None])
    wmap["be2"] = np.ascontiguousarray(be2[:, None])

    in_maps = []
    for k in range(N_CORES):
        sl = slice(k * NODES_PC, (k + 1) * NODES_PC)
        m = dict(wmap)
        m["xT"] = np.ascontiguousarray(gnn_in[sl].T)
        m["c3"] = np.ascontiguousarray(
            np.concatenate([centers[sl].T, np.ones((1, NODES_PC), np.float32)], 0))
        in_maps.append(m)

    kw = {}
    if _trace:
        kw = dict(trace=True, tmpdir=_tmpdir)
    res = bass_utils.run_bass_kernel_spmd(nc, in_maps, core_ids=list(range(N_CORES)), **kw)

    out = np.empty((N, F), np.float32)
    for k in range(N_CORES):
        out[k * NODES_PC:(k + 1) * NODES_PC] = res.results[k]["yT"].T
    if _trace:
        _CACHE["last_res"] = res
    return out
